# revision 13
# baseline (speedup 1.0000x reference)
"""Trainium2 Bass kernel for nn_CatConLayers (multi-head cross-attention over
time/category embeddings).

Sharding: 8 cores = 4 batches x 2 head-pairs. Each core computes, for its
batch b and heads {2g, 2g+1}: hk/hq projections of k_in^T / q_in^T,
scores^T = hk_h^T-chunks @ hq_h, exp (softmax numerator; scores are tiny so
no max-subtraction is needed), the value matmul accumulated over key chunks,
the softmax denominator via a ones-vector matmul, normalization, and the
per-head output projection with Wo. Host: builds k_in^T/q_in^T featurization
(sinusoidal time embedding + category-embedding rows; the ACT Sin table
cannot be co-resident with the Exp table, and on-device indirect-DMA gathers
measured 1.1us each), shards inputs, sums the two head-pair partials per
batch, adds bo.

Matmul operands on the scores path are bf16 (fp32 PSUM accumulation); the
value/output path dtype is selectable (fp32 default for accuracy).

The KQ dimension is permuted (sin block | cos block | emb0 | emb1) so the
interleaved sin/cos layout of the reference never has to be materialized
on-chip; Wq/Wk rows and q_in^T are permuted identically on host.
"""

import numpy as np
import ml_dtypes

import concourse.bass as bass
import concourse.mybir as mybir
import concourse.tile as tile
from concourse import bacc
from concourse.bass_utils import run_bass_kernel_spmd

# Problem shapes (hardcoded per harness contract)
N, T, H, KQ, LD, NREF, DT = 4, 1024, 4, 128, 128, 128, 64
NCORES = 8
TCH = T // 128  # 8 key chunks of 128

F32 = mybir.dt.float32
BF16 = mybir.dt.bfloat16
FP16 = mybir.dt.float16
AF = mybir.ActivationFunctionType

# matmul operand dtype scheme: "f16" = fp16 everywhere (1-pass matmuls,
# ~5e-4 absmax-rel), "hybrid" = bf16 scores + fp32 value (~6e-5, slower)
VALUE_DTYPE = "f16"

_CACHE = {}


def _build_program(vd_name):
    if vd_name == "f16":
        SD = VD = FP16
    elif vd_name == "bf16":
        SD = VD = BF16
    else:  # hybrid
        SD, VD = BF16, F32
    nc = bacc.Bacc("TRN2", target_bir_lowering=False, debug=False,
                   num_devices=NCORES)

    # inputs packed into three blobs, one per DMA queue:
    #   qblob: [qT | wq | wkT | bq2] (scalar engine; gates the first matmuls)
    #   kT: keys-transposed (sync engine)
    #   xblob: [x rearranged | wo] (gpsimd engine)
    kT_d = nc.dram_tensor("kT", [KQ, T], SD, kind="ExternalInput")
    qb_d = nc.dram_tensor("qblob", [128, 642], SD, kind="ExternalInput")
    xb_d = nc.dram_tensor("xblob", [128, T + 2 * LD], VD, kind="ExternalInput")
    out_d = nc.dram_tensor("out", [NREF, LD], F32, kind="ExternalOutput")

    inv_sqrt_kq = float(1.0 / np.sqrt(KQ))

    with tile.TileContext(nc) as tc:
        with tc.tile_pool(name="const", bufs=1) as cp, \
             tc.tile_pool(name="work", bufs=2) as sp, \
             tc.tile_pool(name="ps", bufs=2, space="PSUM") as pp:

            ones_col = cp.tile([128, 1], VD)
            nc.vector.memset(ones_col[:], 1.0)
            one11 = cp.tile([1, 1], F32)
            nc.vector.memset(one11[:], 1.0)

            qblob = cp.tile([128, 642], SD)
            nc.scalar.dma_start(out=qblob[:, 0:384], in_=qb_d[:, 0:384])
            nc.scalar.dma_start(out=qblob[:, 384:642], in_=qb_d[:, 384:642])
            kT = cp.tile([KQ, T], SD)
            nc.sync.dma_start(out=kT[:], in_=kT_d[:])
            xblob = cp.tile([128, T + 2 * LD], VD)
            nc.gpsimd.dma_start(out=xblob[:], in_=xb_d[:])
            bq_sb = sp.tile([KQ, 2], F32, tag="bq", bufs=1)
            nc.vector.tensor_copy(out=bq_sb[:], in_=qblob[:, 640:642])

            # ---- hq^T both heads -> one PSUM tile, bias-add per head.
            qp = pp.tile([128, 2 * NREF], F32, tag="s1", bufs=4)
            for h in range(2):
                nc.tensor.matmul(out=qp[:, h * 128:(h + 1) * 128],
                                 lhsT=qblob[:, h * 128:(h + 1) * 128],
                                 rhs=qblob[:, 256:384], start=True, stop=True)
            hqs = sp.tile([128, 2 * NREF], SD, tag="hqs", bufs=1)
            for h in range(2):
                nc.vector.tensor_scalar_add(out=hqs[:, h * 128:(h + 1) * 128],
                                            in0=qp[:, h * 128:(h + 1) * 128],
                                            scalar1=bq_sb[:, h:h + 1])

            # ---- m_h = Wk_h @ hq_h^T, both heads side by side. scores^T =
            # k_in^T-chunks(stationary) @ [m_0|m_1]; the bk cross-term is
            # constant over keys and cancels exactly in the softmax.
            mp = pp.tile([128, 2 * NREF], F32, tag="s1", bufs=4)
            for h in range(2):
                nc.tensor.matmul(out=mp[:, h * 128:(h + 1) * 128],
                                 lhsT=qblob[:, 384 + h * 128:384 + (h + 1) * 128],
                                 rhs=hqs[:, h * 128:(h + 1) * 128],
                                 start=True, stop=True)
            ms = sp.tile([128, 2 * NREF], SD, tag="ms", bufs=1)
            nc.vector.tensor_copy(out=ms[:], in_=mp[:])

            # ---- scores^T + exp. p~^T layout: chunk c, head h at
            # pT_all[:, c*256 + h*128 ...] so value/Z matmuls batch heads.
            pT_all = cp.tile([128, 2 * T], VD)
            for c2 in range(TCH // 2):
                sc = pp.tile([128, 512], F32, tag="s1", bufs=4)
                for j in range(2):
                    c = c2 * 2 + j
                    nc.tensor.matmul(out=sc[:, j * 256:(j + 1) * 256],
                                     lhsT=kT[:, c * 128:(c + 1) * 128],
                                     rhs=ms[:], start=True, stop=True)
                nc.scalar.activation(
                    out=pT_all[:, c2 * 512:(c2 + 1) * 512],
                    in_=sc[:], func=AF.Exp, scale=inv_sqrt_kq)

            # ---- softmax denominators: partial Z rows via ones^T @ p~T
            # (4 matmuls of N=512), reduced on DVE, transposed to columns
            # by tiny matmuls, then 1/Z.
            zrow = pp.tile([1, 512], F32, tag="s1", bufs=4)
            for c2 in range(TCH // 2):
                nc.tensor.matmul(out=zrow[:],
                                 lhsT=ones_col[:],
                                 rhs=pT_all[:, c2 * 512:(c2 + 1) * 512],
                                 start=(c2 == 0), stop=(c2 == TCH // 2 - 1))
            # zrow col u*256 + h*128 + q holds a 2-chunk partial sum
            zr_sb = sp.tile([1, 2 * NREF], F32, tag="zrs", bufs=1)
            zv = zrow[:].rearrange("p (u h q) -> p h q u", u=2, h=2)
            for h in range(2):
                nc.vector.reduce_sum(out=zr_sb[:, h * 128:(h + 1) * 128],
                                     in_=zv[:, h], axis=mybir.AxisListType.X)
            rinv = []
            for h in range(2):
                zc_ps = pp.tile([NREF, 1], F32, tag="s1", bufs=4)
                nc.tensor.matmul(out=zc_ps[:],
                                 lhsT=zr_sb[:, h * 128:(h + 1) * 128],
                                 rhs=one11[:], start=True, stop=True)
                ri = sp.tile([NREF, 1], F32, tag="ri", bufs=2)
                nc.vector.reciprocal(out=ri[:], in_=zc_ps[:])
                rinv.append(ri)

            # ---- value matmul: vo[v, c-interleaved q pairs] accumulated
            # over the 8 key chunks; both heads per matmul.
            vo = pp.tile([128, 2 * NREF], F32, tag="w2", bufs=2)
            for c in range(TCH):
                nc.tensor.matmul(out=vo[:],
                                 lhsT=xblob[:, c * 128:(c + 1) * 128],
                                 rhs=pT_all[:, c * 256:(c + 1) * 256],
                                 start=(c == 0), stop=(c == TCH - 1))

            # ---- output projection per head, then normalize+combine
            ot = sp.tile([128, 2 * NREF], VD, tag="ots", bufs=1)
            nc.scalar.copy(out=ot[:], in_=vo[:])
            fin = pp.tile([NREF, 2 * LD], F32, tag="s1", bufs=4)
            for h in range(2):
                nc.tensor.matmul(out=fin[:, h * 128:(h + 1) * 128],
                                 lhsT=ot[:, h * 128:(h + 1) * 128],
                                 rhs=xblob[:, T + h * LD:T + (h + 1) * LD],
                                 start=True, stop=True)
            res0 = sp.tile([NREF, LD], F32, tag="res0", bufs=1)
            res1 = sp.tile([NREF, LD], F32, tag="res1", bufs=1)
            nc.vector.tensor_scalar_mul(out=res0[:], in0=fin[:, 0:128],
                                        scalar1=rinv[0][:, :1])
            nc.scalar.activation(out=res1[:], in_=fin[:, 128:256],
                                 func=AF.Copy, scale=rinv[1][:, :1])
            nc.vector.tensor_add(out=res0[:], in0=res0[:], in1=res1[:])
            nc.sync.dma_start(out=out_d[:], in_=res0[:])

    nc.compile()
    return nc


def _get_program(vd_name=None):
    vd_name = vd_name or VALUE_DTYPE
    if vd_name not in _CACHE:
        _CACHE[vd_name] = _build_program(vd_name)
    return _CACHE[vd_name]


def _host_prep(ts, ys0, ys1, emb0, emb1):
    """Full k_in^T (permuted) per batch and q_in^T."""
    div = np.exp(np.arange(0, DT, 2, dtype=np.float32)
                 * (-np.log(10.0) / DT)).astype(np.float32)  # (32,)
    ang = 48.0 * ts[:, :, None].astype(np.float32) * div[None, None, :]
    kT = np.empty((N, KQ, T), np.float32)
    kT[:, 0:32] = np.sin(ang).transpose(0, 2, 1)
    kT[:, 32:64] = np.cos(ang).transpose(0, 2, 1)
    kT[:, 64:96] = emb0[ys0].transpose(0, 2, 1)
    kT[:, 96:128] = emb1[ys1].transpose(0, 2, 1)

    ref = np.linspace(0.0, 1.0, NREF, dtype=np.float32)
    ang_r = 48.0 * ref[:, None] * div[None, :]  # (NREF, 32)
    qT = np.empty((KQ, NREF), np.float32)
    qT[0:32] = np.sin(ang_r).T
    qT[32:64] = np.cos(ang_r).T
    qT[64:96] = emb0[100][:, None]
    qT[96:128] = emb1[50][:, None]
    return kT, qT


def _make_in_maps(ts, ys0, ys1, x, emb0, emb1, Wq, bq, Wk, bk, Wo, vd_name):
    if vd_name == "f16":
        sd = vd = np.float16
    elif vd_name == "bf16":
        sd = vd = ml_dtypes.bfloat16
    else:  # hybrid
        sd, vd = ml_dtypes.bfloat16, np.float32
    bf = sd
    ts = np.asarray(ts, np.float32)
    x = np.asarray(x, np.float32)
    emb0 = np.asarray(emb0, np.float32)
    emb1 = np.asarray(emb1, np.float32)
    ys0 = np.asarray(ys0).astype(np.int64)
    ys1 = np.asarray(ys1).astype(np.int64)

    kT, qT = _host_prep(ts, ys0, ys1, emb0, emb1)
    # KQ permutation: (sin block | cos block | emb0 | emb1) -> reference order
    perm = np.concatenate([2 * np.arange(32), 2 * np.arange(32) + 1,
                           64 + np.arange(32), 96 + np.arange(32)])
    Wq_p = np.asarray(Wq, np.float32)[perm]
    Wk_p = np.asarray(Wk, np.float32)[perm]
    bq2 = np.asarray(bq, np.float32).reshape(H, KQ)
    bk2 = np.asarray(bk, np.float32).reshape(H, KQ)
    Wo = np.asarray(Wo, np.float32)
    # x rearranged: chunk c on cols [c*128,(c+1)*128), key t=c*128+p on part p
    xr = np.ascontiguousarray(
        x.reshape(N, TCH, 128, LD).transpose(0, 2, 1, 3).reshape(N, 128, T))

    in_maps = []
    for c in range(NCORES):
        b, hg = c // 2, c % 2
        # wo laid out (LD, 2*LD): local head h rows at cols [h*LD,(h+1)*LD)
        wo2 = np.ascontiguousarray(
            Wo[hg * 256:(hg + 1) * 256, :].reshape(2, LD, LD)
            .transpose(1, 0, 2).reshape(LD, 2 * LD))
        wkT2 = (Wk_p[:, hg * 256:(hg + 1) * 256].reshape(KQ, 2, KQ)
                .transpose(2, 1, 0).reshape(KQ, 2 * KQ))
        qblob = np.concatenate(
            [Wq_p[:, hg * 256:(hg + 1) * 256], qT, wkT2,
             bq2[2 * hg:2 * hg + 2].T], axis=1)
        xblob = np.concatenate([xr[b], wo2], axis=1)
        in_maps.append(dict(
            kT=kT[b].astype(bf),
            qblob=np.ascontiguousarray(qblob).astype(bf),
            xblob=np.ascontiguousarray(xblob).astype(vd),
        ))
    return in_maps


def kernel(ts, ys0, ys1, x, emb0, emb1, Wq, bq, Wk, bk, Wo, bo):
    in_maps = _make_in_maps(ts, ys0, ys1, x, emb0, emb1, Wq, bq, Wk, bk, Wo,
                            VALUE_DTYPE)
    nc = _get_program()
    res = run_bass_kernel_spmd(nc, in_maps, list(range(NCORES)))
    bo = np.asarray(bo, np.float32)
    out = np.empty((N, NREF, LD), np.float32)
    for b in range(N):
        out[b] = (res.results[2 * b]["out"] + res.results[2 * b + 1]["out"]
                  + bo[None, :])
    return out


# revision 14
# speedup vs baseline: 1.0277x; 1.0277x over previous
"""Trainium2 Bass kernel for nn_CatConLayers (multi-head cross-attention over
time/category embeddings).

Sharding: 8 cores = 4 batches x 2 head-pairs. Each core computes, for its
batch b and heads {2g, 2g+1}: hk/hq projections of k_in^T / q_in^T,
scores^T = hk_h^T-chunks @ hq_h, exp (softmax numerator; scores are tiny so
no max-subtraction is needed), the value matmul accumulated over key chunks,
the softmax denominator via a ones-vector matmul, normalization, and the
per-head output projection with Wo. Host: builds k_in^T/q_in^T featurization
(sinusoidal time embedding + category-embedding rows; the ACT Sin table
cannot be co-resident with the Exp table, and on-device indirect-DMA gathers
measured 1.1us each), shards inputs, sums the two head-pair partials per
batch, adds bo.

Matmul operands on the scores path are bf16 (fp32 PSUM accumulation); the
value/output path dtype is selectable (fp32 default for accuracy).

The KQ dimension is permuted (sin block | cos block | emb0 | emb1) so the
interleaved sin/cos layout of the reference never has to be materialized
on-chip; Wq/Wk rows and q_in^T are permuted identically on host.
"""

import numpy as np
import ml_dtypes

import concourse.bass as bass
import concourse.mybir as mybir
import concourse.tile as tile
from concourse import bacc
from concourse.bass_utils import run_bass_kernel_spmd

# Problem shapes (hardcoded per harness contract)
N, T, H, KQ, LD, NREF, DT = 4, 1024, 4, 128, 128, 128, 64
NCORES = 8
TCH = T // 128  # 8 key chunks of 128

F32 = mybir.dt.float32
BF16 = mybir.dt.bfloat16
FP16 = mybir.dt.float16
AF = mybir.ActivationFunctionType

# matmul operand dtype scheme: "f16" = fp16 everywhere (1-pass matmuls,
# ~5e-4 absmax-rel), "hybrid" = bf16 scores + fp32 value (~6e-5, slower)
VALUE_DTYPE = "f16"

_CACHE = {}


def _build_program(vd_name):
    if vd_name == "f16":
        SD = VD = FP16
    elif vd_name == "bf16":
        SD = VD = BF16
    else:  # hybrid
        SD, VD = BF16, F32
    nc = bacc.Bacc("TRN2", target_bir_lowering=False, debug=False,
                   num_devices=NCORES)

    # inputs packed into three blobs, one per DMA queue:
    #   qblob: [qT | wq | wkT | bq2] (scalar engine; gates the first matmuls)
    #   kT: keys-transposed (sync engine)
    #   xblob: [x rearranged | wo] (gpsimd engine)
    kT_d = nc.dram_tensor("kT", [KQ, T], SD, kind="ExternalInput")
    qb_d = nc.dram_tensor("qblob", [128, 642], SD, kind="ExternalInput")
    xb_d = nc.dram_tensor("xblob", [128, T + 2 * LD], VD, kind="ExternalInput")
    out_d = nc.dram_tensor("out", [NREF, LD], F32, kind="ExternalOutput")

    inv_sqrt_kq = float(1.0 / np.sqrt(KQ))

    with tile.TileContext(nc) as tc:
        with tc.tile_pool(name="const", bufs=1) as cp, \
             tc.tile_pool(name="work", bufs=2) as sp, \
             tc.tile_pool(name="ps", bufs=2, space="PSUM") as pp:

            ones_col = cp.tile([128, 1], VD)
            nc.vector.memset(ones_col[:], 1.0)
            one11 = cp.tile([1, 1], F32)
            nc.vector.memset(one11[:], 1.0)

            qblob = cp.tile([128, 642], SD)
            nc.scalar.dma_start(out=qblob[:], in_=qb_d[:])
            kT = cp.tile([KQ, T], SD)
            nc.sync.dma_start(out=kT[:], in_=kT_d[:])
            xblob = cp.tile([128, T + 2 * LD], VD)
            nc.gpsimd.dma_start(out=xblob[:], in_=xb_d[:])
            bq_sb = sp.tile([KQ, 2], F32, tag="bq", bufs=1)
            nc.vector.tensor_copy(out=bq_sb[:], in_=qblob[:, 640:642])

            # ---- hq^T both heads -> one PSUM tile, bias-add per head.
            qp = pp.tile([128, 2 * NREF], F32, tag="s1", bufs=4)
            for h in range(2):
                nc.tensor.matmul(out=qp[:, h * 128:(h + 1) * 128],
                                 lhsT=qblob[:, h * 128:(h + 1) * 128],
                                 rhs=qblob[:, 256:384], start=True, stop=True)
            hqs = sp.tile([128, 2 * NREF], SD, tag="hqs", bufs=1)
            for h in range(2):
                nc.vector.tensor_scalar_add(out=hqs[:, h * 128:(h + 1) * 128],
                                            in0=qp[:, h * 128:(h + 1) * 128],
                                            scalar1=bq_sb[:, h:h + 1])

            # ---- m_h = Wk_h @ hq_h^T, both heads side by side. scores^T =
            # k_in^T-chunks(stationary) @ [m_0|m_1]; the bk cross-term is
            # constant over keys and cancels exactly in the softmax.
            mp = pp.tile([128, 2 * NREF], F32, tag="s1", bufs=4)
            for h in range(2):
                nc.tensor.matmul(out=mp[:, h * 128:(h + 1) * 128],
                                 lhsT=qblob[:, 384 + h * 128:384 + (h + 1) * 128],
                                 rhs=hqs[:, h * 128:(h + 1) * 128],
                                 start=True, stop=True)
            ms = sp.tile([128, 2 * NREF], SD, tag="ms", bufs=1)
            nc.vector.tensor_copy(out=ms[:], in_=mp[:])

            # ---- scores^T + exp. p~^T layout: chunk c, head h at
            # pT_all[:, c*256 + h*128 ...] so value/Z matmuls batch heads.
            pT_all = cp.tile([128, 2 * T], VD)
            for c2 in range(TCH // 2):
                sc = pp.tile([128, 512], F32, tag="s1", bufs=4)
                for j in range(2):
                    c = c2 * 2 + j
                    nc.tensor.matmul(out=sc[:, j * 256:(j + 1) * 256],
                                     lhsT=kT[:, c * 128:(c + 1) * 128],
                                     rhs=ms[:], start=True, stop=True)
                nc.scalar.activation(
                    out=pT_all[:, c2 * 512:(c2 + 1) * 512],
                    in_=sc[:], func=AF.Exp, scale=inv_sqrt_kq)

            # ---- softmax denominators: partial Z rows via ones^T @ p~T
            # (4 matmuls of N=512), reduced on DVE, transposed to columns
            # by tiny matmuls, then 1/Z.
            zrow = pp.tile([1, 512], F32, tag="s1", bufs=4)
            for c2 in range(TCH // 2):
                nc.tensor.matmul(out=zrow[:],
                                 lhsT=ones_col[:],
                                 rhs=pT_all[:, c2 * 512:(c2 + 1) * 512],
                                 start=(c2 == 0), stop=(c2 == TCH // 2 - 1))
            # zrow col u*256 + h*128 + q holds a 2-chunk partial sum
            zr_sb = sp.tile([1, 2 * NREF], F32, tag="zrs", bufs=1)
            zv = zrow[:].rearrange("p (u h q) -> p h q u", u=2, h=2)
            for h in range(2):
                nc.vector.reduce_sum(out=zr_sb[:, h * 128:(h + 1) * 128],
                                     in_=zv[:, h], axis=mybir.AxisListType.X)
            rinv = []
            for h in range(2):
                zc_ps = pp.tile([NREF, 1], F32, tag="s1", bufs=4)
                nc.tensor.matmul(out=zc_ps[:],
                                 lhsT=zr_sb[:, h * 128:(h + 1) * 128],
                                 rhs=one11[:], start=True, stop=True)
                ri = sp.tile([NREF, 1], F32, tag="ri", bufs=2)
                nc.vector.reciprocal(out=ri[:], in_=zc_ps[:])
                rinv.append(ri)

            # ---- value matmul: vo[v, c-interleaved q pairs] accumulated
            # over the 8 key chunks; both heads per matmul.
            vo = pp.tile([128, 2 * NREF], F32, tag="w2", bufs=2)
            for c in range(TCH):
                nc.tensor.matmul(out=vo[:],
                                 lhsT=xblob[:, c * 128:(c + 1) * 128],
                                 rhs=pT_all[:, c * 256:(c + 1) * 256],
                                 start=(c == 0), stop=(c == TCH - 1))

            # ---- output projection per head, then normalize+combine
            ot = sp.tile([128, 2 * NREF], VD, tag="ots", bufs=1)
            nc.scalar.copy(out=ot[:], in_=vo[:])
            fin = pp.tile([NREF, 2 * LD], F32, tag="s1", bufs=4)
            for h in range(2):
                nc.tensor.matmul(out=fin[:, h * 128:(h + 1) * 128],
                                 lhsT=ot[:, h * 128:(h + 1) * 128],
                                 rhs=xblob[:, T + h * LD:T + (h + 1) * LD],
                                 start=True, stop=True)
            res0 = sp.tile([NREF, LD], F32, tag="res0", bufs=1)
            res1 = sp.tile([NREF, LD], F32, tag="res1", bufs=1)
            nc.vector.tensor_scalar_mul(out=res0[:], in0=fin[:, 0:128],
                                        scalar1=rinv[0][:, :1])
            nc.scalar.activation(out=res1[:], in_=fin[:, 128:256],
                                 func=AF.Copy, scale=rinv[1][:, :1])
            nc.vector.tensor_add(out=res0[:], in0=res0[:], in1=res1[:])
            nc.sync.dma_start(out=out_d[:], in_=res0[:])

    nc.compile()
    return nc


def _get_program(vd_name=None):
    vd_name = vd_name or VALUE_DTYPE
    if vd_name not in _CACHE:
        _CACHE[vd_name] = _build_program(vd_name)
    return _CACHE[vd_name]


def _host_prep(ts, ys0, ys1, emb0, emb1):
    """Full k_in^T (permuted) per batch and q_in^T."""
    div = np.exp(np.arange(0, DT, 2, dtype=np.float32)
                 * (-np.log(10.0) / DT)).astype(np.float32)  # (32,)
    ang = 48.0 * ts[:, :, None].astype(np.float32) * div[None, None, :]
    kT = np.empty((N, KQ, T), np.float32)
    kT[:, 0:32] = np.sin(ang).transpose(0, 2, 1)
    kT[:, 32:64] = np.cos(ang).transpose(0, 2, 1)
    kT[:, 64:96] = emb0[ys0].transpose(0, 2, 1)
    kT[:, 96:128] = emb1[ys1].transpose(0, 2, 1)

    ref = np.linspace(0.0, 1.0, NREF, dtype=np.float32)
    ang_r = 48.0 * ref[:, None] * div[None, :]  # (NREF, 32)
    qT = np.empty((KQ, NREF), np.float32)
    qT[0:32] = np.sin(ang_r).T
    qT[32:64] = np.cos(ang_r).T
    qT[64:96] = emb0[100][:, None]
    qT[96:128] = emb1[50][:, None]
    return kT, qT


def _make_in_maps(ts, ys0, ys1, x, emb0, emb1, Wq, bq, Wk, bk, Wo, vd_name):
    if vd_name == "f16":
        sd = vd = np.float16
    elif vd_name == "bf16":
        sd = vd = ml_dtypes.bfloat16
    else:  # hybrid
        sd, vd = ml_dtypes.bfloat16, np.float32
    bf = sd
    ts = np.asarray(ts, np.float32)
    x = np.asarray(x, np.float32)
    emb0 = np.asarray(emb0, np.float32)
    emb1 = np.asarray(emb1, np.float32)
    ys0 = np.asarray(ys0).astype(np.int64)
    ys1 = np.asarray(ys1).astype(np.int64)

    kT, qT = _host_prep(ts, ys0, ys1, emb0, emb1)
    # KQ permutation: (sin block | cos block | emb0 | emb1) -> reference order
    perm = np.concatenate([2 * np.arange(32), 2 * np.arange(32) + 1,
                           64 + np.arange(32), 96 + np.arange(32)])
    Wq_p = np.asarray(Wq, np.float32)[perm]
    Wk_p = np.asarray(Wk, np.float32)[perm]
    bq2 = np.asarray(bq, np.float32).reshape(H, KQ)
    bk2 = np.asarray(bk, np.float32).reshape(H, KQ)
    Wo = np.asarray(Wo, np.float32)
    # x rearranged: chunk c on cols [c*128,(c+1)*128), key t=c*128+p on part p
    xr = np.ascontiguousarray(
        x.reshape(N, TCH, 128, LD).transpose(0, 2, 1, 3).reshape(N, 128, T))

    in_maps = []
    for c in range(NCORES):
        b, hg = c // 2, c % 2
        # wo laid out (LD, 2*LD): local head h rows at cols [h*LD,(h+1)*LD)
        wo2 = np.ascontiguousarray(
            Wo[hg * 256:(hg + 1) * 256, :].reshape(2, LD, LD)
            .transpose(1, 0, 2).reshape(LD, 2 * LD))
        wkT2 = (Wk_p[:, hg * 256:(hg + 1) * 256].reshape(KQ, 2, KQ)
                .transpose(2, 1, 0).reshape(KQ, 2 * KQ))
        qblob = np.concatenate(
            [Wq_p[:, hg * 256:(hg + 1) * 256], qT, wkT2,
             bq2[2 * hg:2 * hg + 2].T], axis=1)
        xblob = np.concatenate([xr[b], wo2], axis=1)
        in_maps.append(dict(
            kT=kT[b].astype(bf),
            qblob=np.ascontiguousarray(qblob).astype(bf),
            xblob=np.ascontiguousarray(xblob).astype(vd),
        ))
    return in_maps


def kernel(ts, ys0, ys1, x, emb0, emb1, Wq, bq, Wk, bk, Wo, bo):
    in_maps = _make_in_maps(ts, ys0, ys1, x, emb0, emb1, Wq, bq, Wk, bk, Wo,
                            VALUE_DTYPE)
    nc = _get_program()
    res = run_bass_kernel_spmd(nc, in_maps, list(range(NCORES)))
    bo = np.asarray(bo, np.float32)
    out = np.empty((N, NREF, LD), np.float32)
    for b in range(N):
        out[b] = (res.results[2 * b]["out"] + res.results[2 * b + 1]["out"]
                  + bo[None, :])
    return out


# revision 18
# speedup vs baseline: 1.0856x; 1.0564x over previous
"""Trainium2 Bass kernel for nn_CatConLayers (multi-head cross-attention over
time/category embeddings).

Sharding: 8 cores = 4 batches x 2 head-pairs. Each core computes, for its
batch b and heads {2g, 2g+1}: hk/hq projections of k_in^T / q_in^T,
scores^T = hk_h^T-chunks @ hq_h, exp (softmax numerator; scores are tiny so
no max-subtraction is needed), the value matmul accumulated over key chunks,
the softmax denominator via a ones-vector matmul, normalization, and the
per-head output projection with Wo. Host: builds k_in^T/q_in^T featurization
(sinusoidal time embedding + category-embedding rows; the ACT Sin table
cannot be co-resident with the Exp table, and on-device indirect-DMA gathers
measured 1.1us each), shards inputs, sums the two head-pair partials per
batch, adds bo.

Matmul operands on the scores path are bf16 (fp32 PSUM accumulation); the
value/output path dtype is selectable (fp32 default for accuracy).

The KQ dimension is permuted (sin block | cos block | emb0 | emb1) so the
interleaved sin/cos layout of the reference never has to be materialized
on-chip; Wq/Wk rows and q_in^T are permuted identically on host.
"""

import numpy as np
import ml_dtypes

import concourse.bass as bass
import concourse.mybir as mybir
import concourse.tile as tile
from concourse import bacc
from concourse.bass_utils import run_bass_kernel_spmd

# Problem shapes (hardcoded per harness contract)
N, T, H, KQ, LD, NREF, DT = 4, 1024, 4, 128, 128, 128, 64
NCORES = 8
TCH = T // 128  # 8 key chunks of 128

F32 = mybir.dt.float32
BF16 = mybir.dt.bfloat16
FP16 = mybir.dt.float16
AF = mybir.ActivationFunctionType

# matmul operand dtype scheme: "f16" = fp16 everywhere (1-pass matmuls,
# ~5e-4 absmax-rel), "hybrid" = bf16 scores + fp32 value (~6e-5, slower)
VALUE_DTYPE = "f16"

_CACHE = {}


def _build_program(vd_name):
    if vd_name == "f16":
        SD = VD = FP16
    elif vd_name == "bf16":
        SD = VD = BF16
    else:  # hybrid
        SD, VD = BF16, F32
    nc = bacc.Bacc("TRN2", target_bir_lowering=False, debug=False,
                   num_devices=NCORES)

    # inputs packed into three blobs, one per DMA queue:
    #   qblob: [qT | wq | wkT | bq2] (scalar engine; gates the first matmuls)
    #   kT: keys-transposed (sync engine)
    #   xblob: [x rearranged | wo] (gpsimd engine)
    kT_d = nc.dram_tensor("kT", [KQ, T], SD, kind="ExternalInput")
    qb_d = nc.dram_tensor("qblob", [128, 642], SD, kind="ExternalInput")
    xb_d = nc.dram_tensor("xblob", [128, T + 2 * LD], VD, kind="ExternalInput")
    out_d = nc.dram_tensor("out", [NREF, LD], F32, kind="ExternalOutput")

    inv_sqrt_kq = float(1.0 / np.sqrt(KQ))

    with tile.TileContext(nc) as tc:
        with tc.tile_pool(name="const", bufs=1) as cp, \
             tc.tile_pool(name="work", bufs=2) as sp, \
             tc.tile_pool(name="ps", bufs=2, space="PSUM") as pp:

            ones_col = cp.tile([128, 1], VD)
            nc.vector.memset(ones_col[:], 1.0)
            one11 = cp.tile([1, 1], F32)
            nc.vector.memset(one11[:], 1.0)

            qblob = cp.tile([128, 642], SD)
            nc.scalar.dma_start(out=qblob[:], in_=qb_d[:])
            kT = cp.tile([KQ, T], SD)
            nc.sync.dma_start(out=kT[:], in_=kT_d[:])
            xblob = cp.tile([128, T + 2 * LD], VD)
            nc.gpsimd.dma_start(out=xblob[:], in_=xb_d[:])
            bq_sb = sp.tile([KQ, 2], F32, tag="bq", bufs=1)
            nc.vector.tensor_copy(out=bq_sb[:], in_=qblob[:, 640:642])

            # ---- hq^T both heads -> one PSUM tile, bias-add per head.
            qp = pp.tile([128, 2 * NREF], F32, tag="s1", bufs=4)
            for h in range(2):
                nc.tensor.matmul(out=qp[:, h * 128:(h + 1) * 128],
                                 lhsT=qblob[:, h * 128:(h + 1) * 128],
                                 rhs=qblob[:, 256:384], start=True, stop=True)
            hqs = sp.tile([128, 2 * NREF], SD, tag="hqs", bufs=1)
            for h in range(2):
                nc.vector.tensor_scalar_add(out=hqs[:, h * 128:(h + 1) * 128],
                                            in0=qp[:, h * 128:(h + 1) * 128],
                                            scalar1=bq_sb[:, h:h + 1])

            # ---- m_h = Wk_h @ hq_h^T, both heads side by side. scores^T =
            # k_in^T-chunks(stationary) @ [m_0|m_1]; the bk cross-term is
            # constant over keys and cancels exactly in the softmax.
            mp = pp.tile([128, 2 * NREF], F32, tag="s1", bufs=4)
            for h in range(2):
                nc.tensor.matmul(out=mp[:, h * 128:(h + 1) * 128],
                                 lhsT=qblob[:, 384 + h * 128:384 + (h + 1) * 128],
                                 rhs=hqs[:, h * 128:(h + 1) * 128],
                                 start=True, stop=True)
            ms = sp.tile([128, 2 * NREF], SD, tag="ms", bufs=1)
            nc.vector.tensor_copy(out=ms[:], in_=mp[:])

            # ---- scores^T + exp. p~^T layout: chunk c, head h at
            # pT_all[:, c*256 + h*128 ...] so value/Z matmuls batch heads.
            pT_all = cp.tile([128, 2 * T], VD)
            for c2 in range(TCH // 2):
                sc = pp.tile([128, 512], F32, tag="s1", bufs=4)
                for j in range(2):
                    c = c2 * 2 + j
                    nc.tensor.matmul(out=sc[:, j * 256:(j + 1) * 256],
                                     lhsT=kT[:, c * 128:(c + 1) * 128],
                                     rhs=ms[:], start=True, stop=True)
                nc.scalar.activation(
                    out=pT_all[:, c2 * 512:(c2 + 1) * 512],
                    in_=sc[:], func=AF.Exp, scale=inv_sqrt_kq)

            # ---- softmax denominators: partial Z rows via ones^T @ p~T
            # (4 matmuls of N=512), reduced on DVE, transposed to columns
            # by tiny matmuls, then 1/Z.
            zrow = pp.tile([1, 512], F32, tag="s1", bufs=4)
            for c2 in range(TCH // 2):
                nc.tensor.matmul(out=zrow[:],
                                 lhsT=ones_col[:],
                                 rhs=pT_all[:, c2 * 512:(c2 + 1) * 512],
                                 start=(c2 == 0), stop=(c2 == TCH // 2 - 1))
            # zrow col u*256 + h*128 + q holds a 2-chunk partial sum
            zr_sb = sp.tile([1, 2 * NREF], F32, tag="zrs", bufs=1)
            zv = zrow[:].rearrange("p (u h q) -> p h q u", u=2, h=2)
            for h in range(2):
                nc.vector.reduce_sum(out=zr_sb[:, h * 128:(h + 1) * 128],
                                     in_=zv[:, h], axis=mybir.AxisListType.X)
            rinv = []
            for h in range(2):
                zc_ps = pp.tile([NREF, 1], F32, tag="s1", bufs=4)
                nc.tensor.matmul(out=zc_ps[:],
                                 lhsT=zr_sb[:, h * 128:(h + 1) * 128],
                                 rhs=one11[:], start=True, stop=True)
                ri = sp.tile([NREF, 1], F32, tag="ri", bufs=2)
                nc.vector.reciprocal(out=ri[:], in_=zc_ps[:])
                rinv.append(ri)

            # ---- value matmul: vo[v, c-interleaved q pairs] accumulated
            # over the 8 key chunks; both heads per matmul.
            vo = pp.tile([128, 2 * NREF], F32, tag="w2", bufs=2)
            for c in range(TCH):
                nc.tensor.matmul(out=vo[:],
                                 lhsT=xblob[:, c * 128:(c + 1) * 128],
                                 rhs=pT_all[:, c * 256:(c + 1) * 256],
                                 start=(c == 0), stop=(c == TCH - 1))

            # ---- output projection per head, then normalize+combine
            ot = sp.tile([128, 2 * NREF], VD, tag="ots", bufs=1)
            nc.scalar.copy(out=ot[:], in_=vo[:])
            fin = pp.tile([NREF, 2 * LD], F32, tag="s1", bufs=4)
            for h in range(2):
                nc.tensor.matmul(out=fin[:, h * 128:(h + 1) * 128],
                                 lhsT=ot[:, h * 128:(h + 1) * 128],
                                 rhs=xblob[:, T + h * LD:T + (h + 1) * LD],
                                 start=True, stop=True)
            res0 = sp.tile([NREF, LD], F32, tag="res0", bufs=1)
            res1 = sp.tile([NREF, LD], F32, tag="res1", bufs=1)
            nc.vector.tensor_scalar_mul(out=res0[:], in0=fin[:, 0:128],
                                        scalar1=rinv[0][:, :1])
            nc.scalar.activation(out=res1[:], in_=fin[:, 128:256],
                                 func=AF.Copy, scale=rinv[1][:, :1])
            nc.vector.tensor_add(out=res0[:], in0=res0[:], in1=res1[:])
            nc.sync.dma_start(out=out_d[:], in_=res0[:])

    nc.compile()
    return nc


def _build_program_raw(vd_name):
    """Raw bacc (no TileContext): manual semaphores, no kernel-tail barrier."""
    assert vd_name == "f16"
    SD = VD = FP16
    nc = bacc.Bacc("TRN2", target_bir_lowering=False, debug=False,
                   num_devices=NCORES)

    kT_d = nc.dram_tensor("kT", [KQ, T], SD, kind="ExternalInput")
    qb_d = nc.dram_tensor("qblob", [128, 642], SD, kind="ExternalInput")
    xb_d = nc.dram_tensor("xblob", [128, T + 2 * LD], VD, kind="ExternalInput")
    out_d = nc.dram_tensor("out", [NREF, LD], F32, kind="ExternalOutput")
    inv_sqrt_kq = float(1.0 / np.sqrt(KQ))

    from contextlib import ExitStack
    st = ExitStack()
    sb = lambda shape, dt, name: nc.alloc_sbuf_tensor(name, list(shape), dt).ap()
    qblob = sb([128, 642], SD, "qblob_sb")
    kT = sb([KQ, T], SD, "kT_sb")
    xblob = sb([128, T + 2 * LD], VD, "xblob_sb")
    bq_sb = sb([KQ, 2], F32, "bq_sb")
    hqs = sb([128, 2 * NREF], SD, "hqs_sb")
    ms = sb([128, 2 * NREF], SD, "ms_sb")
    pT = sb([128, 2 * T], VD, "pT_sb")
    zr_sb = sb([1, 2 * NREF], F32, "zr_sb")
    ri0 = sb([NREF, 1], F32, "ri0_sb")
    ri1 = sb([NREF, 1], F32, "ri1_sb")
    ot = sb([128, 2 * NREF], VD, "ot_sb")
    res0 = sb([NREF, LD], F32, "res0_sb")
    res1 = sb([NREF, LD], F32, "res1_sb")
    ones_col = sb([128, 1], VD, "ones_sb")
    one11 = sb([1, 1], F32, "one11_sb")

    qp = st.enter_context(nc.psum_tensor("qp_ps", [128, 2 * NREF], F32))
    mp = st.enter_context(nc.psum_tensor("mp_ps", [128, 2 * NREF], F32))
    scs = [st.enter_context(nc.psum_tensor(f"sc{i}_ps", [128, 512], F32)) for i in range(2)]
    vo = st.enter_context(nc.psum_tensor("vo_ps", [128, 2 * NREF], F32))
    zrow = st.enter_context(nc.psum_tensor("zrow_ps", [1, 512], F32))
    zc = st.enter_context(nc.psum_tensor("zc_ps", [NREF, 2], F32))
    fin = st.enter_context(nc.psum_tensor("fin_ps", [NREF, 2 * LD], F32))

    with nc.Block() as block, \
         nc.semaphore("dq") as dq, nc.semaphore("dk") as dk, \
         nc.semaphore("dx") as dx, nc.semaphore("s_pe") as s_pe, \
         nc.semaphore("s_dve") as s_dve, nc.semaphore("s_act") as s_act, \
         nc.semaphore("s_out") as s_out:

        # PE sem counts: hq:1,2  m:3,4  sc:5..12  z/val per c2: z,v,v ->
        # 13,14,15 | 16,17,18 | 19,20,21 | 22,23,24  zc:25,26  fin:27,28
        # DVE: bqcast:1 bias:2,3 ms:4 zred:5,6 recip:7,8 res0:9 add:10
        # ACT: exp:1..4 ot:5 res1:6

        @block.scalar
        def _(act):
            act.dma_start(out=qblob[:], in_=qb_d[:]).then_inc(dq, 16)
            for c2 in range(4):
                act.wait_ge(s_pe, 5 + 2 * (c2 + 1) - 1)  # scores pair done
                act.activation(out=pT[:, c2 * 512:(c2 + 1) * 512],
                               in_=scs[c2 % 2][:], func=AF.Exp,
                               scale=inv_sqrt_kq).then_inc(s_act, 1)
            act.wait_ge(s_pe, 24)  # vo accumulation complete
            act.activation(out=ot[:], in_=vo[:],
                           func=AF.Copy).then_inc(s_act, 1)
            act.wait_ge(s_pe, 28)  # fin1 done
            act.wait_ge(s_dve, 8)  # recip1 done
            act.activation(out=res1[:], in_=fin[:, 128:256], func=AF.Copy,
                           scale=ri1[:, :1]).then_inc(s_act, 1)

        @block.sync
        def _(sync):
            sync.dma_start(out=kT[:], in_=kT_d[:]).then_inc(dk, 16)
            sync.wait_ge(s_dve, 10)
            sync.dma_start(out=out_d[:], in_=res0[:]).then_inc(s_out, 16)
            sync.wait_ge(s_out, 16)

        @block.gpsimd
        def _(g):
            g.dma_start(out=xblob[:], in_=xb_d[:]).then_inc(dx, 16)

        @block.vector
        def _(v):
            v.memset(ones_col[:], 1.0)
            v.memset(one11[:], 1.0)
            v.wait_ge(dq, 16)
            v.tensor_copy(out=bq_sb[:], in_=qblob[:, 640:642]).then_inc(s_dve, 1)
            v.wait_ge(s_pe, 2)
            for h in range(2):
                v.tensor_scalar_add(out=hqs[:, h * 128:(h + 1) * 128],
                                    in0=qp[:, h * 128:(h + 1) * 128],
                                    scalar1=bq_sb[:, h:h + 1]).then_inc(s_dve, 1)
            v.wait_ge(s_pe, 4)
            v.tensor_copy(out=ms[:], in_=mp[:]).then_inc(s_dve, 1)
            v.wait_ge(s_pe, 22)  # all 4 z MMs done (counts 13,16,19,22)
            zv = zrow[:].rearrange("p (u h q) -> p h q u", u=2, h=2)
            for h in range(2):
                v.reduce_sum(out=zr_sb[:, h * 128:(h + 1) * 128], in_=zv[:, h],
                             axis=mybir.AxisListType.X).then_inc(s_dve, 1)
            v.wait_ge(s_pe, 25)
            v.reciprocal(out=ri0[:], in_=zc[:, 0:1]).then_inc(s_dve, 1)
            v.wait_ge(s_pe, 26)
            v.reciprocal(out=ri1[:], in_=zc[:, 1:2]).then_inc(s_dve, 1)
            v.wait_ge(s_pe, 27)
            v.tensor_scalar_mul(out=res0[:], in0=fin[:, 0:128],
                                scalar1=ri0[:, :1]).then_inc(s_dve, 1)
            v.wait_ge(s_act, 6)
            v.tensor_add(out=res0[:], in0=res0[:],
                         in1=res1[:]).then_inc(s_dve, 1)

        @block.tensor
        def _(t):
            t.wait_ge(dq, 16)
            for h in range(2):
                t.matmul(out=qp[:, h * 128:(h + 1) * 128],
                         lhsT=qblob[:, h * 128:(h + 1) * 128],
                         rhs=qblob[:, 256:384], start=True,
                         stop=True).then_inc(s_pe, 1)
            t.wait_ge(s_dve, 3)
            for h in range(2):
                t.matmul(out=mp[:, h * 128:(h + 1) * 128],
                         lhsT=qblob[:, 384 + h * 128:384 + (h + 1) * 128],
                         rhs=hqs[:, h * 128:(h + 1) * 128], start=True,
                         stop=True).then_inc(s_pe, 1)
            t.wait_ge(s_dve, 4)
            t.wait_ge(dk, 16)
            for c2 in range(4):
                if c2 >= 2:
                    t.wait_ge(s_act, c2 - 1)
                for j in range(2):
                    c = c2 * 2 + j
                    t.matmul(out=scs[c2 % 2][:, j * 256:(j + 1) * 256],
                             lhsT=kT[:, c * 128:(c + 1) * 128],
                             rhs=ms[:], start=True,
                             stop=True).then_inc(s_pe, 1)
            t.wait_ge(dx, 16)
            for c2 in range(4):
                t.wait_ge(s_act, c2 + 1)
                t.matmul(out=zrow[:], lhsT=ones_col[:],
                         rhs=pT[:, c2 * 512:(c2 + 1) * 512],
                         start=(c2 == 0), stop=(c2 == 3),
                         skip_group_check=True).then_inc(s_pe, 1)
                for j in range(2):
                    c = c2 * 2 + j
                    t.matmul(out=vo[:],
                             lhsT=xblob[:, c * 128:(c + 1) * 128],
                             rhs=pT[:, c * 256:(c + 1) * 256],
                             start=(c == 0), stop=(c == 7),
                             skip_group_check=True).then_inc(s_pe, 1)
            t.wait_ge(s_dve, 5)
            t.matmul(out=zc[:, 0:1], lhsT=zr_sb[:, 0:128], rhs=one11[:],
                     start=True, stop=True).then_inc(s_pe, 1)
            t.wait_ge(s_dve, 6)
            t.matmul(out=zc[:, 1:2], lhsT=zr_sb[:, 128:256], rhs=one11[:],
                     start=True, stop=True).then_inc(s_pe, 1)
            t.wait_ge(s_act, 5)
            for h in range(2):
                t.matmul(out=fin[:, h * 128:(h + 1) * 128],
                         lhsT=ot[:, h * 128:(h + 1) * 128],
                         rhs=xblob[:, T + h * LD:T + (h + 1) * LD],
                         start=True, stop=True).then_inc(s_pe, 1)

    st.close()
    nc.compile()
    return nc


USE_RAW = True


def _get_program(vd_name=None):
    vd_name = vd_name or VALUE_DTYPE
    key = ("raw" if USE_RAW else "tile") + vd_name
    if key not in _CACHE:
        builder = _build_program_raw if USE_RAW else _build_program
        _CACHE[key] = builder(vd_name)
    return _CACHE[key]


def _host_prep(ts, ys0, ys1, emb0, emb1):
    """Full k_in^T (permuted) per batch and q_in^T."""
    div = np.exp(np.arange(0, DT, 2, dtype=np.float32)
                 * (-np.log(10.0) / DT)).astype(np.float32)  # (32,)
    ang = 48.0 * ts[:, :, None].astype(np.float32) * div[None, None, :]
    kT = np.empty((N, KQ, T), np.float32)
    kT[:, 0:32] = np.sin(ang).transpose(0, 2, 1)
    kT[:, 32:64] = np.cos(ang).transpose(0, 2, 1)
    kT[:, 64:96] = emb0[ys0].transpose(0, 2, 1)
    kT[:, 96:128] = emb1[ys1].transpose(0, 2, 1)

    ref = np.linspace(0.0, 1.0, NREF, dtype=np.float32)
    ang_r = 48.0 * ref[:, None] * div[None, :]  # (NREF, 32)
    qT = np.empty((KQ, NREF), np.float32)
    qT[0:32] = np.sin(ang_r).T
    qT[32:64] = np.cos(ang_r).T
    qT[64:96] = emb0[100][:, None]
    qT[96:128] = emb1[50][:, None]
    return kT, qT


def _make_in_maps(ts, ys0, ys1, x, emb0, emb1, Wq, bq, Wk, bk, Wo, vd_name):
    if vd_name == "f16":
        sd = vd = np.float16
    elif vd_name == "bf16":
        sd = vd = ml_dtypes.bfloat16
    else:  # hybrid
        sd, vd = ml_dtypes.bfloat16, np.float32
    bf = sd
    ts = np.asarray(ts, np.float32)
    x = np.asarray(x, np.float32)
    emb0 = np.asarray(emb0, np.float32)
    emb1 = np.asarray(emb1, np.float32)
    ys0 = np.asarray(ys0).astype(np.int64)
    ys1 = np.asarray(ys1).astype(np.int64)

    kT, qT = _host_prep(ts, ys0, ys1, emb0, emb1)
    # KQ permutation: (sin block | cos block | emb0 | emb1) -> reference order
    perm = np.concatenate([2 * np.arange(32), 2 * np.arange(32) + 1,
                           64 + np.arange(32), 96 + np.arange(32)])
    Wq_p = np.asarray(Wq, np.float32)[perm]
    Wk_p = np.asarray(Wk, np.float32)[perm]
    bq2 = np.asarray(bq, np.float32).reshape(H, KQ)
    bk2 = np.asarray(bk, np.float32).reshape(H, KQ)
    Wo = np.asarray(Wo, np.float32)
    # x rearranged: chunk c on cols [c*128,(c+1)*128), key t=c*128+p on part p
    xr = np.ascontiguousarray(
        x.reshape(N, TCH, 128, LD).transpose(0, 2, 1, 3).reshape(N, 128, T))

    in_maps = []
    for c in range(NCORES):
        b, hg = c // 2, c % 2
        # wo laid out (LD, 2*LD): local head h rows at cols [h*LD,(h+1)*LD)
        wo2 = np.ascontiguousarray(
            Wo[hg * 256:(hg + 1) * 256, :].reshape(2, LD, LD)
            .transpose(1, 0, 2).reshape(LD, 2 * LD))
        wkT2 = (Wk_p[:, hg * 256:(hg + 1) * 256].reshape(KQ, 2, KQ)
                .transpose(2, 1, 0).reshape(KQ, 2 * KQ))
        qblob = np.concatenate(
            [Wq_p[:, hg * 256:(hg + 1) * 256], qT, wkT2,
             bq2[2 * hg:2 * hg + 2].T], axis=1)
        xblob = np.concatenate([xr[b], wo2], axis=1)
        in_maps.append(dict(
            kT=kT[b].astype(bf),
            qblob=np.ascontiguousarray(qblob).astype(bf),
            xblob=np.ascontiguousarray(xblob).astype(vd),
        ))
    return in_maps


def kernel(ts, ys0, ys1, x, emb0, emb1, Wq, bq, Wk, bk, Wo, bo):
    in_maps = _make_in_maps(ts, ys0, ys1, x, emb0, emb1, Wq, bq, Wk, bk, Wo,
                            VALUE_DTYPE)
    nc = _get_program()
    res = run_bass_kernel_spmd(nc, in_maps, list(range(NCORES)))
    bo = np.asarray(bo, np.float32)
    out = np.empty((N, NREF, LD), np.float32)
    for b in range(N):
        out[b] = (res.results[2 * b]["out"] + res.results[2 * b + 1]["out"]
                  + bo[None, :])
    return out


# revision 19
# speedup vs baseline: 1.1197x; 1.0314x over previous
"""Trainium2 Bass kernel for nn_CatConLayers (multi-head cross-attention over
time/category embeddings).

Sharding: 8 cores = 4 batches x 2 head-pairs. Each core computes, for its
batch b and heads {2g, 2g+1}: hk/hq projections of k_in^T / q_in^T,
scores^T = hk_h^T-chunks @ hq_h, exp (softmax numerator; scores are tiny so
no max-subtraction is needed), the value matmul accumulated over key chunks,
the softmax denominator via a ones-vector matmul, normalization, and the
per-head output projection with Wo. Host: builds k_in^T/q_in^T featurization
(sinusoidal time embedding + category-embedding rows; the ACT Sin table
cannot be co-resident with the Exp table, and on-device indirect-DMA gathers
measured 1.1us each), shards inputs, sums the two head-pair partials per
batch, adds bo.

Matmul operands on the scores path are bf16 (fp32 PSUM accumulation); the
value/output path dtype is selectable (fp32 default for accuracy).

The KQ dimension is permuted (sin block | cos block | emb0 | emb1) so the
interleaved sin/cos layout of the reference never has to be materialized
on-chip; Wq/Wk rows and q_in^T are permuted identically on host.
"""

import numpy as np
import ml_dtypes

import concourse.bass as bass
import concourse.mybir as mybir
import concourse.tile as tile
from concourse import bacc
from concourse.bass_utils import run_bass_kernel_spmd

# Problem shapes (hardcoded per harness contract)
N, T, H, KQ, LD, NREF, DT = 4, 1024, 4, 128, 128, 128, 64
NCORES = 8
TCH = T // 128  # 8 key chunks of 128

F32 = mybir.dt.float32
BF16 = mybir.dt.bfloat16
FP16 = mybir.dt.float16
AF = mybir.ActivationFunctionType

# matmul operand dtype scheme: "f16" = fp16 everywhere (1-pass matmuls,
# ~5e-4 absmax-rel), "hybrid" = bf16 scores + fp32 value (~6e-5, slower)
VALUE_DTYPE = "f16"

_CACHE = {}


def _build_program(vd_name):
    if vd_name == "f16":
        SD = VD = FP16
    elif vd_name == "bf16":
        SD = VD = BF16
    else:  # hybrid
        SD, VD = BF16, F32
    nc = bacc.Bacc("TRN2", target_bir_lowering=False, debug=False,
                   num_devices=NCORES)

    # inputs packed into three blobs, one per DMA queue:
    #   qblob: [qT | wq | wkT | bq2] (scalar engine; gates the first matmuls)
    #   kT: keys-transposed (sync engine)
    #   xblob: [x rearranged | wo] (gpsimd engine)
    kT_d = nc.dram_tensor("kT", [KQ, T], SD, kind="ExternalInput")
    qb_d = nc.dram_tensor("qblob", [128, 642], SD, kind="ExternalInput")
    xb_d = nc.dram_tensor("xblob", [128, T + 2 * LD], VD, kind="ExternalInput")
    out_d = nc.dram_tensor("out", [NREF, LD], F32, kind="ExternalOutput")

    inv_sqrt_kq = float(1.0 / np.sqrt(KQ))

    with tile.TileContext(nc) as tc:
        with tc.tile_pool(name="const", bufs=1) as cp, \
             tc.tile_pool(name="work", bufs=2) as sp, \
             tc.tile_pool(name="ps", bufs=2, space="PSUM") as pp:

            ones_col = cp.tile([128, 1], VD)
            nc.vector.memset(ones_col[:], 1.0)
            one11 = cp.tile([1, 1], F32)
            nc.vector.memset(one11[:], 1.0)

            qblob = cp.tile([128, 642], SD)
            nc.scalar.dma_start(out=qblob[:, 0:384], in_=qb_d[:, 0:384])
            nc.sync.dma_start(out=qblob[:, 384:642], in_=qb_d[:, 384:642])
            kT = cp.tile([KQ, T], SD)
            nc.sync.dma_start(out=kT[:], in_=kT_d[:])
            xblob = cp.tile([128, T + 2 * LD], VD)
            nc.gpsimd.dma_start(out=xblob[:], in_=xb_d[:])
            bq_sb = sp.tile([KQ, 2], F32, tag="bq", bufs=1)
            nc.vector.tensor_copy(out=bq_sb[:], in_=qblob[:, 640:642])

            # ---- hq^T both heads -> one PSUM tile, bias-add per head.
            qp = pp.tile([128, 2 * NREF], F32, tag="s1", bufs=4)
            for h in range(2):
                nc.tensor.matmul(out=qp[:, h * 128:(h + 1) * 128],
                                 lhsT=qblob[:, h * 128:(h + 1) * 128],
                                 rhs=qblob[:, 256:384], start=True, stop=True)
            hqs = sp.tile([128, 2 * NREF], SD, tag="hqs", bufs=1)
            for h in range(2):
                nc.vector.tensor_scalar_add(out=hqs[:, h * 128:(h + 1) * 128],
                                            in0=qp[:, h * 128:(h + 1) * 128],
                                            scalar1=bq_sb[:, h:h + 1])

            # ---- m_h = Wk_h @ hq_h^T, both heads side by side. scores^T =
            # k_in^T-chunks(stationary) @ [m_0|m_1]; the bk cross-term is
            # constant over keys and cancels exactly in the softmax.
            mp = pp.tile([128, 2 * NREF], F32, tag="s1", bufs=4)
            for h in range(2):
                nc.tensor.matmul(out=mp[:, h * 128:(h + 1) * 128],
                                 lhsT=qblob[:, 384 + h * 128:384 + (h + 1) * 128],
                                 rhs=hqs[:, h * 128:(h + 1) * 128],
                                 start=True, stop=True)
            ms = sp.tile([128, 2 * NREF], SD, tag="ms", bufs=1)
            nc.vector.tensor_copy(out=ms[:], in_=mp[:])

            # ---- scores^T + exp. p~^T layout: chunk c, head h at
            # pT_all[:, c*256 + h*128 ...] so value/Z matmuls batch heads.
            pT_all = cp.tile([128, 2 * T], VD)
            for c2 in range(TCH // 2):
                sc = pp.tile([128, 512], F32, tag="s1", bufs=4)
                for j in range(2):
                    c = c2 * 2 + j
                    nc.tensor.matmul(out=sc[:, j * 256:(j + 1) * 256],
                                     lhsT=kT[:, c * 128:(c + 1) * 128],
                                     rhs=ms[:], start=True, stop=True)
                nc.scalar.activation(
                    out=pT_all[:, c2 * 512:(c2 + 1) * 512],
                    in_=sc[:], func=AF.Exp, scale=inv_sqrt_kq)

            # ---- softmax denominators: partial Z rows via ones^T @ p~T
            # (4 matmuls of N=512), reduced on DVE, transposed to columns
            # by tiny matmuls, then 1/Z.
            zrow = pp.tile([1, 512], F32, tag="s1", bufs=4)
            for c2 in range(TCH // 2):
                nc.tensor.matmul(out=zrow[:],
                                 lhsT=ones_col[:],
                                 rhs=pT_all[:, c2 * 512:(c2 + 1) * 512],
                                 start=(c2 == 0), stop=(c2 == TCH // 2 - 1))
            # zrow col u*256 + h*128 + q holds a 2-chunk partial sum
            zr_sb = sp.tile([1, 2 * NREF], F32, tag="zrs", bufs=1)
            zv = zrow[:].rearrange("p (u h q) -> p h q u", u=2, h=2)
            for h in range(2):
                nc.vector.reduce_sum(out=zr_sb[:, h * 128:(h + 1) * 128],
                                     in_=zv[:, h], axis=mybir.AxisListType.X)
            rinv = []
            for h in range(2):
                zc_ps = pp.tile([NREF, 1], F32, tag="s1", bufs=4)
                nc.tensor.matmul(out=zc_ps[:],
                                 lhsT=zr_sb[:, h * 128:(h + 1) * 128],
                                 rhs=one11[:], start=True, stop=True)
                ri = sp.tile([NREF, 1], F32, tag="ri", bufs=2)
                nc.vector.reciprocal(out=ri[:], in_=zc_ps[:])
                rinv.append(ri)

            # ---- value matmul: vo[v, c-interleaved q pairs] accumulated
            # over the 8 key chunks; both heads per matmul.
            vo = pp.tile([128, 2 * NREF], F32, tag="w2", bufs=2)
            for c in range(TCH):
                nc.tensor.matmul(out=vo[:],
                                 lhsT=xblob[:, c * 128:(c + 1) * 128],
                                 rhs=pT_all[:, c * 256:(c + 1) * 256],
                                 start=(c == 0), stop=(c == TCH - 1))

            # ---- output projection per head, then normalize+combine
            ot = sp.tile([128, 2 * NREF], VD, tag="ots", bufs=1)
            nc.scalar.copy(out=ot[:], in_=vo[:])
            fin = pp.tile([NREF, 2 * LD], F32, tag="s1", bufs=4)
            for h in range(2):
                nc.tensor.matmul(out=fin[:, h * 128:(h + 1) * 128],
                                 lhsT=ot[:, h * 128:(h + 1) * 128],
                                 rhs=xblob[:, T + h * LD:T + (h + 1) * LD],
                                 start=True, stop=True)
            res0 = sp.tile([NREF, LD], F32, tag="res0", bufs=1)
            res1 = sp.tile([NREF, LD], F32, tag="res1", bufs=1)
            nc.vector.tensor_scalar_mul(out=res0[:], in0=fin[:, 0:128],
                                        scalar1=rinv[0][:, :1])
            nc.scalar.activation(out=res1[:], in_=fin[:, 128:256],
                                 func=AF.Copy, scale=rinv[1][:, :1])
            nc.vector.tensor_add(out=res0[:], in0=res0[:], in1=res1[:])
            nc.sync.dma_start(out=out_d[:], in_=res0[:])

    nc.compile()
    return nc


def _build_program_raw(vd_name):
    """Raw bacc (no TileContext): manual semaphores, no kernel-tail barrier."""
    assert vd_name == "f16"
    SD = VD = FP16
    nc = bacc.Bacc("TRN2", target_bir_lowering=False, debug=False,
                   num_devices=NCORES)

    kT_d = nc.dram_tensor("kT", [KQ, T], SD, kind="ExternalInput")
    qb_d = nc.dram_tensor("qblob", [128, 642], SD, kind="ExternalInput")
    xb_d = nc.dram_tensor("xblob", [128, T + 2 * LD], VD, kind="ExternalInput")
    out_d = nc.dram_tensor("out", [NREF, LD], F32, kind="ExternalOutput")
    inv_sqrt_kq = float(1.0 / np.sqrt(KQ))

    from contextlib import ExitStack
    st = ExitStack()
    sb = lambda shape, dt, name: nc.alloc_sbuf_tensor(name, list(shape), dt).ap()
    qblob = sb([128, 642], SD, "qblob_sb")
    kT = sb([KQ, T], SD, "kT_sb")
    xblob = sb([128, T + 2 * LD], VD, "xblob_sb")
    bq_sb = sb([KQ, 2], F32, "bq_sb")
    hqs = sb([128, 2 * NREF], SD, "hqs_sb")
    ms = sb([128, 2 * NREF], SD, "ms_sb")
    pT = sb([128, 2 * T], VD, "pT_sb")
    zr_sb = sb([1, 2 * NREF], F32, "zr_sb")
    ri0 = sb([NREF, 1], F32, "ri0_sb")
    ri1 = sb([NREF, 1], F32, "ri1_sb")
    ot = sb([128, 2 * NREF], VD, "ot_sb")
    res0 = sb([NREF, LD], F32, "res0_sb")
    res1 = sb([NREF, LD], F32, "res1_sb")
    ones_col = sb([128, 1], VD, "ones_sb")
    one11 = sb([1, 1], F32, "one11_sb")

    qp = st.enter_context(nc.psum_tensor("qp_ps", [128, 2 * NREF], F32))
    mp = st.enter_context(nc.psum_tensor("mp_ps", [128, 2 * NREF], F32))
    scs = [st.enter_context(nc.psum_tensor(f"sc{i}_ps", [128, 512], F32)) for i in range(2)]
    vo = st.enter_context(nc.psum_tensor("vo_ps", [128, 2 * NREF], F32))
    zrow = st.enter_context(nc.psum_tensor("zrow_ps", [1, 512], F32))
    zc = st.enter_context(nc.psum_tensor("zc_ps", [NREF, 2], F32))
    fin = st.enter_context(nc.psum_tensor("fin_ps", [NREF, 2 * LD], F32))

    with nc.Block() as block, \
         nc.semaphore("dq") as dq, nc.semaphore("dk") as dk, \
         nc.semaphore("dx") as dx, nc.semaphore("s_pe") as s_pe, \
         nc.semaphore("s_dve") as s_dve, nc.semaphore("s_act") as s_act, \
         nc.semaphore("s_out") as s_out:

        # PE sem counts: hq:1,2  m:3,4  sc:5..12  z/val per c2: z,v,v ->
        # 13,14,15 | 16,17,18 | 19,20,21 | 22,23,24  zc:25,26  fin:27,28
        # DVE: bqcast:1 bias:2,3 ms:4 zred:5,6 recip:7,8 res0:9 add:10
        # ACT: exp:1..4 ot:5 res1:6

        @block.scalar
        def _(act):
            act.dma_start(out=qblob[:], in_=qb_d[:]).then_inc(dq, 16)
            for c2 in range(4):
                act.wait_ge(s_pe, 5 + 2 * (c2 + 1) - 1)  # scores pair done
                act.activation(out=pT[:, c2 * 512:(c2 + 1) * 512],
                               in_=scs[c2 % 2][:], func=AF.Exp,
                               scale=inv_sqrt_kq).then_inc(s_act, 1)
            act.wait_ge(s_pe, 24)  # vo accumulation complete
            act.activation(out=ot[:], in_=vo[:],
                           func=AF.Copy).then_inc(s_act, 1)
            act.wait_ge(s_pe, 28)  # fin1 done
            act.wait_ge(s_dve, 8)  # recip1 done
            act.activation(out=res1[:], in_=fin[:, 128:256], func=AF.Copy,
                           scale=ri1[:, :1]).then_inc(s_act, 1)

        @block.sync
        def _(sync):
            sync.dma_start(out=kT[:], in_=kT_d[:]).then_inc(dk, 16)
            sync.wait_ge(s_dve, 10)
            sync.dma_start(out=out_d[:], in_=res0[:]).then_inc(s_out, 16)
            sync.wait_ge(s_out, 16)

        @block.gpsimd
        def _(g):
            g.dma_start(out=xblob[:], in_=xb_d[:]).then_inc(dx, 16)

        @block.vector
        def _(v):
            v.memset(ones_col[:], 1.0)
            v.memset(one11[:], 1.0)
            v.wait_ge(dq, 16)
            v.tensor_copy(out=bq_sb[:], in_=qblob[:, 640:642]).then_inc(s_dve, 1)
            v.wait_ge(s_pe, 2)
            for h in range(2):
                v.tensor_scalar_add(out=hqs[:, h * 128:(h + 1) * 128],
                                    in0=qp[:, h * 128:(h + 1) * 128],
                                    scalar1=bq_sb[:, h:h + 1]).then_inc(s_dve, 1)
            v.wait_ge(s_pe, 4)
            v.tensor_copy(out=ms[:], in_=mp[:]).then_inc(s_dve, 1)
            v.wait_ge(s_pe, 22)  # all 4 z MMs done (counts 13,16,19,22)
            zv = zrow[:].rearrange("p (u h q) -> p h q u", u=2, h=2)
            for h in range(2):
                v.reduce_sum(out=zr_sb[:, h * 128:(h + 1) * 128], in_=zv[:, h],
                             axis=mybir.AxisListType.X).then_inc(s_dve, 1)
            v.wait_ge(s_pe, 25)
            v.reciprocal(out=ri0[:], in_=zc[:, 0:1]).then_inc(s_dve, 1)
            v.wait_ge(s_pe, 26)
            v.reciprocal(out=ri1[:], in_=zc[:, 1:2]).then_inc(s_dve, 1)
            v.wait_ge(s_pe, 27)
            v.tensor_scalar_mul(out=res0[:], in0=fin[:, 0:128],
                                scalar1=ri0[:, :1]).then_inc(s_dve, 1)
            v.wait_ge(s_act, 6)
            v.tensor_add(out=res0[:], in0=res0[:],
                         in1=res1[:]).then_inc(s_dve, 1)

        @block.tensor
        def _(t):
            t.wait_ge(dq, 16)
            for h in range(2):
                t.matmul(out=qp[:, h * 128:(h + 1) * 128],
                         lhsT=qblob[:, h * 128:(h + 1) * 128],
                         rhs=qblob[:, 256:384], start=True,
                         stop=True).then_inc(s_pe, 1)
            t.wait_ge(s_dve, 3)
            for h in range(2):
                t.matmul(out=mp[:, h * 128:(h + 1) * 128],
                         lhsT=qblob[:, 384 + h * 128:384 + (h + 1) * 128],
                         rhs=hqs[:, h * 128:(h + 1) * 128], start=True,
                         stop=True).then_inc(s_pe, 1)
            t.wait_ge(s_dve, 4)
            t.wait_ge(dk, 16)
            for c2 in range(4):
                if c2 >= 2:
                    t.wait_ge(s_act, c2 - 1)
                for j in range(2):
                    c = c2 * 2 + j
                    t.matmul(out=scs[c2 % 2][:, j * 256:(j + 1) * 256],
                             lhsT=kT[:, c * 128:(c + 1) * 128],
                             rhs=ms[:], start=True,
                             stop=True).then_inc(s_pe, 1)
            t.wait_ge(dx, 16)
            for c2 in range(4):
                t.wait_ge(s_act, c2 + 1)
                t.matmul(out=zrow[:], lhsT=ones_col[:],
                         rhs=pT[:, c2 * 512:(c2 + 1) * 512],
                         start=(c2 == 0), stop=(c2 == 3),
                         skip_group_check=True).then_inc(s_pe, 1)
                for j in range(2):
                    c = c2 * 2 + j
                    t.matmul(out=vo[:],
                             lhsT=xblob[:, c * 128:(c + 1) * 128],
                             rhs=pT[:, c * 256:(c + 1) * 256],
                             start=(c == 0), stop=(c == 7),
                             skip_group_check=True).then_inc(s_pe, 1)
            t.wait_ge(s_dve, 5)
            t.matmul(out=zc[:, 0:1], lhsT=zr_sb[:, 0:128], rhs=one11[:],
                     start=True, stop=True).then_inc(s_pe, 1)
            t.wait_ge(s_dve, 6)
            t.matmul(out=zc[:, 1:2], lhsT=zr_sb[:, 128:256], rhs=one11[:],
                     start=True, stop=True).then_inc(s_pe, 1)
            t.wait_ge(s_act, 5)
            for h in range(2):
                t.matmul(out=fin[:, h * 128:(h + 1) * 128],
                         lhsT=ot[:, h * 128:(h + 1) * 128],
                         rhs=xblob[:, T + h * LD:T + (h + 1) * LD],
                         start=True, stop=True).then_inc(s_pe, 1)

    st.close()
    nc.compile()
    return nc


USE_RAW = False


def _get_program(vd_name=None):
    vd_name = vd_name or VALUE_DTYPE
    key = ("raw" if USE_RAW else "tile") + vd_name
    if key not in _CACHE:
        builder = _build_program_raw if USE_RAW else _build_program
        _CACHE[key] = builder(vd_name)
    return _CACHE[key]


def _host_prep(ts, ys0, ys1, emb0, emb1):
    """Full k_in^T (permuted) per batch and q_in^T."""
    div = np.exp(np.arange(0, DT, 2, dtype=np.float32)
                 * (-np.log(10.0) / DT)).astype(np.float32)  # (32,)
    ang = 48.0 * ts[:, :, None].astype(np.float32) * div[None, None, :]
    kT = np.empty((N, KQ, T), np.float32)
    kT[:, 0:32] = np.sin(ang).transpose(0, 2, 1)
    kT[:, 32:64] = np.cos(ang).transpose(0, 2, 1)
    kT[:, 64:96] = emb0[ys0].transpose(0, 2, 1)
    kT[:, 96:128] = emb1[ys1].transpose(0, 2, 1)

    ref = np.linspace(0.0, 1.0, NREF, dtype=np.float32)
    ang_r = 48.0 * ref[:, None] * div[None, :]  # (NREF, 32)
    qT = np.empty((KQ, NREF), np.float32)
    qT[0:32] = np.sin(ang_r).T
    qT[32:64] = np.cos(ang_r).T
    qT[64:96] = emb0[100][:, None]
    qT[96:128] = emb1[50][:, None]
    return kT, qT


def _make_in_maps(ts, ys0, ys1, x, emb0, emb1, Wq, bq, Wk, bk, Wo, vd_name):
    if vd_name == "f16":
        sd = vd = np.float16
    elif vd_name == "bf16":
        sd = vd = ml_dtypes.bfloat16
    else:  # hybrid
        sd, vd = ml_dtypes.bfloat16, np.float32
    bf = sd
    ts = np.asarray(ts, np.float32)
    x = np.asarray(x, np.float32)
    emb0 = np.asarray(emb0, np.float32)
    emb1 = np.asarray(emb1, np.float32)
    ys0 = np.asarray(ys0).astype(np.int64)
    ys1 = np.asarray(ys1).astype(np.int64)

    kT, qT = _host_prep(ts, ys0, ys1, emb0, emb1)
    # KQ permutation: (sin block | cos block | emb0 | emb1) -> reference order
    perm = np.concatenate([2 * np.arange(32), 2 * np.arange(32) + 1,
                           64 + np.arange(32), 96 + np.arange(32)])
    Wq_p = np.asarray(Wq, np.float32)[perm]
    Wk_p = np.asarray(Wk, np.float32)[perm]
    bq2 = np.asarray(bq, np.float32).reshape(H, KQ)
    bk2 = np.asarray(bk, np.float32).reshape(H, KQ)
    Wo = np.asarray(Wo, np.float32)
    # x rearranged: chunk c on cols [c*128,(c+1)*128), key t=c*128+p on part p
    xr = np.ascontiguousarray(
        x.reshape(N, TCH, 128, LD).transpose(0, 2, 1, 3).reshape(N, 128, T))

    in_maps = []
    for c in range(NCORES):
        b, hg = c // 2, c % 2
        # wo laid out (LD, 2*LD): local head h rows at cols [h*LD,(h+1)*LD)
        wo2 = np.ascontiguousarray(
            Wo[hg * 256:(hg + 1) * 256, :].reshape(2, LD, LD)
            .transpose(1, 0, 2).reshape(LD, 2 * LD))
        wkT2 = (Wk_p[:, hg * 256:(hg + 1) * 256].reshape(KQ, 2, KQ)
                .transpose(2, 1, 0).reshape(KQ, 2 * KQ))
        qblob = np.concatenate(
            [Wq_p[:, hg * 256:(hg + 1) * 256], qT, wkT2,
             bq2[2 * hg:2 * hg + 2].T], axis=1)
        xblob = np.concatenate([xr[b], wo2], axis=1)
        in_maps.append(dict(
            kT=kT[b].astype(bf),
            qblob=np.ascontiguousarray(qblob).astype(bf),
            xblob=np.ascontiguousarray(xblob).astype(vd),
        ))
    return in_maps


def kernel(ts, ys0, ys1, x, emb0, emb1, Wq, bq, Wk, bk, Wo, bo):
    in_maps = _make_in_maps(ts, ys0, ys1, x, emb0, emb1, Wq, bq, Wk, bk, Wo,
                            VALUE_DTYPE)
    nc = _get_program()
    res = run_bass_kernel_spmd(nc, in_maps, list(range(NCORES)))
    bo = np.asarray(bo, np.float32)
    out = np.empty((N, NREF, LD), np.float32)
    for b in range(N):
        out[b] = (res.results[2 * b]["out"] + res.results[2 * b + 1]["out"]
                  + bo[None, :])
    return out


# revision 20
# speedup vs baseline: 1.1709x; 1.0457x over previous
"""Trainium2 Bass kernel for nn_CatConLayers (multi-head cross-attention over
time/category embeddings).

Sharding: 8 cores = 4 batches x 2 head-pairs. Each core computes, for its
batch b and heads {2g, 2g+1}: hk/hq projections of k_in^T / q_in^T,
scores^T = hk_h^T-chunks @ hq_h, exp (softmax numerator; scores are tiny so
no max-subtraction is needed), the value matmul accumulated over key chunks,
the softmax denominator via a ones-vector matmul, normalization, and the
per-head output projection with Wo. Host: builds k_in^T/q_in^T featurization
(sinusoidal time embedding + category-embedding rows; the ACT Sin table
cannot be co-resident with the Exp table, and on-device indirect-DMA gathers
measured 1.1us each), shards inputs, sums the two head-pair partials per
batch, adds bo.

Matmul operands on the scores path are bf16 (fp32 PSUM accumulation); the
value/output path dtype is selectable (fp32 default for accuracy).

The KQ dimension is permuted (sin block | cos block | emb0 | emb1) so the
interleaved sin/cos layout of the reference never has to be materialized
on-chip; Wq/Wk rows and q_in^T are permuted identically on host.
"""

import numpy as np
import ml_dtypes

import concourse.bass as bass
import concourse.mybir as mybir
import concourse.tile as tile
from concourse import bacc
from concourse.bass_utils import run_bass_kernel_spmd

# Problem shapes (hardcoded per harness contract)
N, T, H, KQ, LD, NREF, DT = 4, 1024, 4, 128, 128, 128, 64
NCORES = 8
TCH = T // 128  # 8 key chunks of 128

F32 = mybir.dt.float32
BF16 = mybir.dt.bfloat16
FP16 = mybir.dt.float16
AF = mybir.ActivationFunctionType

# matmul operand dtype scheme: "f16" = fp16 everywhere (1-pass matmuls,
# ~5e-4 absmax-rel), "hybrid" = bf16 scores + fp32 value (~6e-5, slower)
VALUE_DTYPE = "f16"

_CACHE = {}


def _build_program(vd_name):
    if vd_name == "f16":
        SD = VD = FP16
    elif vd_name == "bf16":
        SD = VD = BF16
    else:  # hybrid
        SD, VD = BF16, F32
    nc = bacc.Bacc("TRN2", target_bir_lowering=False, debug=False,
                   num_devices=NCORES)

    # inputs packed into three blobs, one per DMA queue:
    #   qblob: [qT | wq | wkT | bq2] (scalar engine; gates the first matmuls)
    #   kT: keys-transposed (sync engine)
    #   xblob: [x rearranged | wo] (gpsimd engine)
    kT_d = nc.dram_tensor("kT", [KQ, T], SD, kind="ExternalInput")
    qb_d = nc.dram_tensor("qblob", [128, 386], SD, kind="ExternalInput")
    xb_d = nc.dram_tensor("xblob", [128, T + 2 * LD], VD, kind="ExternalInput")
    out_d = nc.dram_tensor("out", [NREF, LD], F32, kind="ExternalOutput")

    inv_sqrt_kq = float(1.0 / np.sqrt(KQ))

    with tile.TileContext(nc) as tc:
        with tc.tile_pool(name="const", bufs=1) as cp, \
             tc.tile_pool(name="work", bufs=2) as sp, \
             tc.tile_pool(name="ps", bufs=2, space="PSUM") as pp:

            ones_col = cp.tile([128, 1], VD)
            nc.vector.memset(ones_col[:], 1.0)
            one11 = cp.tile([1, 1], F32)
            nc.vector.memset(one11[:], 1.0)

            qblob = cp.tile([128, 386], SD)
            nc.scalar.dma_start(out=qblob[:], in_=qb_d[:])
            kT = cp.tile([KQ, T], SD)
            nc.sync.dma_start(out=kT[:], in_=kT_d[:])
            xblob = cp.tile([128, T + 2 * LD], VD)
            nc.gpsimd.dma_start(out=xblob[:], in_=xb_d[:])
            wkbq_sb = sp.tile([KQ, 2], F32, tag="bq", bufs=1)
            nc.vector.tensor_copy(out=wkbq_sb[:], in_=qblob[:, 384:386])

            # ---- m_h = WW_h^T @ q_in^T + Wk_h@bq_h, heads side by side,
            # with WW_h = Wq_h @ Wk_h^T and Wk_h@bq_h fused on host (pure
            # weight preprocessing). scores^T = k_in^T-chunks(stationary) @
            # [m_0|m_1]; the bk cross-term is constant over keys and cancels
            # exactly in the softmax.
            mp = pp.tile([128, 2 * NREF], F32, tag="s1", bufs=4)
            for h in range(2):
                nc.tensor.matmul(out=mp[:, h * 128:(h + 1) * 128],
                                 lhsT=qblob[:, h * 128:(h + 1) * 128],
                                 rhs=qblob[:, 256:384], start=True, stop=True)
            ms = sp.tile([128, 2 * NREF], SD, tag="ms", bufs=1)
            for h in range(2):
                nc.vector.tensor_scalar_add(out=ms[:, h * 128:(h + 1) * 128],
                                            in0=mp[:, h * 128:(h + 1) * 128],
                                            scalar1=wkbq_sb[:, h:h + 1])

            # ---- scores^T + exp. p~^T layout: chunk c, head h at
            # pT_all[:, c*256 + h*128 ...] so value/Z matmuls batch heads.
            pT_all = cp.tile([128, 2 * T], VD)
            for c2 in range(TCH // 2):
                sc = pp.tile([128, 512], F32, tag="s1", bufs=4)
                for j in range(2):
                    c = c2 * 2 + j
                    nc.tensor.matmul(out=sc[:, j * 256:(j + 1) * 256],
                                     lhsT=kT[:, c * 128:(c + 1) * 128],
                                     rhs=ms[:], start=True, stop=True)
                nc.scalar.activation(
                    out=pT_all[:, c2 * 512:(c2 + 1) * 512],
                    in_=sc[:], func=AF.Exp, scale=inv_sqrt_kq)

            # ---- softmax denominators: partial Z rows via ones^T @ p~T
            # (4 matmuls of N=512), reduced on DVE, transposed to columns
            # by tiny matmuls, then 1/Z.
            zrow = pp.tile([1, 512], F32, tag="s1", bufs=4)
            for c2 in range(TCH // 2):
                nc.tensor.matmul(out=zrow[:],
                                 lhsT=ones_col[:],
                                 rhs=pT_all[:, c2 * 512:(c2 + 1) * 512],
                                 start=(c2 == 0), stop=(c2 == TCH // 2 - 1))
            # zrow col u*256 + h*128 + q holds a 2-chunk partial sum
            zr_sb = sp.tile([1, 2 * NREF], F32, tag="zrs", bufs=1)
            zv = zrow[:].rearrange("p (u h q) -> p h q u", u=2, h=2)
            for h in range(2):
                nc.vector.reduce_sum(out=zr_sb[:, h * 128:(h + 1) * 128],
                                     in_=zv[:, h], axis=mybir.AxisListType.X)
            rinv = []
            for h in range(2):
                zc_ps = pp.tile([NREF, 1], F32, tag="s1", bufs=4)
                nc.tensor.matmul(out=zc_ps[:],
                                 lhsT=zr_sb[:, h * 128:(h + 1) * 128],
                                 rhs=one11[:], start=True, stop=True)
                ri = sp.tile([NREF, 1], F32, tag="ri", bufs=2)
                nc.vector.reciprocal(out=ri[:], in_=zc_ps[:])
                rinv.append(ri)

            # ---- value matmul: vo[v, c-interleaved q pairs] accumulated
            # over the 8 key chunks; both heads per matmul.
            vo = pp.tile([128, 2 * NREF], F32, tag="w2", bufs=2)
            for c in range(TCH):
                nc.tensor.matmul(out=vo[:],
                                 lhsT=xblob[:, c * 128:(c + 1) * 128],
                                 rhs=pT_all[:, c * 256:(c + 1) * 256],
                                 start=(c == 0), stop=(c == TCH - 1))

            # ---- output projection per head, then normalize+combine
            ot = sp.tile([128, 2 * NREF], VD, tag="ots", bufs=1)
            nc.scalar.copy(out=ot[:], in_=vo[:])
            fin = pp.tile([NREF, 2 * LD], F32, tag="s1", bufs=4)
            for h in range(2):
                nc.tensor.matmul(out=fin[:, h * 128:(h + 1) * 128],
                                 lhsT=ot[:, h * 128:(h + 1) * 128],
                                 rhs=xblob[:, T + h * LD:T + (h + 1) * LD],
                                 start=True, stop=True)
            res0 = sp.tile([NREF, LD], F32, tag="res0", bufs=1)
            res1 = sp.tile([NREF, LD], F32, tag="res1", bufs=1)
            nc.vector.tensor_scalar_mul(out=res0[:], in0=fin[:, 0:128],
                                        scalar1=rinv[0][:, :1])
            nc.scalar.activation(out=res1[:], in_=fin[:, 128:256],
                                 func=AF.Copy, scale=rinv[1][:, :1])
            nc.vector.tensor_add(out=res0[:], in0=res0[:], in1=res1[:])
            nc.sync.dma_start(out=out_d[:], in_=res0[:])

    nc.compile()
    return nc


def _build_program_raw(vd_name):
    """Raw bacc (no TileContext): manual semaphores, no kernel-tail barrier."""
    assert vd_name == "f16"
    SD = VD = FP16
    nc = bacc.Bacc("TRN2", target_bir_lowering=False, debug=False,
                   num_devices=NCORES)

    kT_d = nc.dram_tensor("kT", [KQ, T], SD, kind="ExternalInput")
    qb_d = nc.dram_tensor("qblob", [128, 386], SD, kind="ExternalInput")
    xb_d = nc.dram_tensor("xblob", [128, T + 2 * LD], VD, kind="ExternalInput")
    out_d = nc.dram_tensor("out", [NREF, LD], F32, kind="ExternalOutput")
    inv_sqrt_kq = float(1.0 / np.sqrt(KQ))

    from contextlib import ExitStack
    st = ExitStack()
    sb = lambda shape, dt, name: nc.alloc_sbuf_tensor(name, list(shape), dt).ap()
    qblob = sb([128, 642], SD, "qblob_sb")
    kT = sb([KQ, T], SD, "kT_sb")
    xblob = sb([128, T + 2 * LD], VD, "xblob_sb")
    bq_sb = sb([KQ, 2], F32, "bq_sb")
    hqs = sb([128, 2 * NREF], SD, "hqs_sb")
    ms = sb([128, 2 * NREF], SD, "ms_sb")
    pT = sb([128, 2 * T], VD, "pT_sb")
    zr_sb = sb([1, 2 * NREF], F32, "zr_sb")
    ri0 = sb([NREF, 1], F32, "ri0_sb")
    ri1 = sb([NREF, 1], F32, "ri1_sb")
    ot = sb([128, 2 * NREF], VD, "ot_sb")
    res0 = sb([NREF, LD], F32, "res0_sb")
    res1 = sb([NREF, LD], F32, "res1_sb")
    ones_col = sb([128, 1], VD, "ones_sb")
    one11 = sb([1, 1], F32, "one11_sb")

    qp = st.enter_context(nc.psum_tensor("qp_ps", [128, 2 * NREF], F32))
    mp = st.enter_context(nc.psum_tensor("mp_ps", [128, 2 * NREF], F32))
    scs = [st.enter_context(nc.psum_tensor(f"sc{i}_ps", [128, 512], F32)) for i in range(2)]
    vo = st.enter_context(nc.psum_tensor("vo_ps", [128, 2 * NREF], F32))
    zrow = st.enter_context(nc.psum_tensor("zrow_ps", [1, 512], F32))
    zc = st.enter_context(nc.psum_tensor("zc_ps", [NREF, 2], F32))
    fin = st.enter_context(nc.psum_tensor("fin_ps", [NREF, 2 * LD], F32))

    with nc.Block() as block, \
         nc.semaphore("dq") as dq, nc.semaphore("dk") as dk, \
         nc.semaphore("dx") as dx, nc.semaphore("s_pe") as s_pe, \
         nc.semaphore("s_dve") as s_dve, nc.semaphore("s_act") as s_act, \
         nc.semaphore("s_out") as s_out:

        # PE sem counts: hq:1,2  m:3,4  sc:5..12  z/val per c2: z,v,v ->
        # 13,14,15 | 16,17,18 | 19,20,21 | 22,23,24  zc:25,26  fin:27,28
        # DVE: bqcast:1 bias:2,3 ms:4 zred:5,6 recip:7,8 res0:9 add:10
        # ACT: exp:1..4 ot:5 res1:6

        @block.scalar
        def _(act):
            act.dma_start(out=qblob[:], in_=qb_d[:]).then_inc(dq, 16)
            for c2 in range(4):
                act.wait_ge(s_pe, 5 + 2 * (c2 + 1) - 1)  # scores pair done
                act.activation(out=pT[:, c2 * 512:(c2 + 1) * 512],
                               in_=scs[c2 % 2][:], func=AF.Exp,
                               scale=inv_sqrt_kq).then_inc(s_act, 1)
            act.wait_ge(s_pe, 24)  # vo accumulation complete
            act.activation(out=ot[:], in_=vo[:],
                           func=AF.Copy).then_inc(s_act, 1)
            act.wait_ge(s_pe, 28)  # fin1 done
            act.wait_ge(s_dve, 8)  # recip1 done
            act.activation(out=res1[:], in_=fin[:, 128:256], func=AF.Copy,
                           scale=ri1[:, :1]).then_inc(s_act, 1)

        @block.sync
        def _(sync):
            sync.dma_start(out=kT[:], in_=kT_d[:]).then_inc(dk, 16)
            sync.wait_ge(s_dve, 10)
            sync.dma_start(out=out_d[:], in_=res0[:]).then_inc(s_out, 16)
            sync.wait_ge(s_out, 16)

        @block.gpsimd
        def _(g):
            g.dma_start(out=xblob[:], in_=xb_d[:]).then_inc(dx, 16)

        @block.vector
        def _(v):
            v.memset(ones_col[:], 1.0)
            v.memset(one11[:], 1.0)
            v.wait_ge(dq, 16)
            v.tensor_copy(out=bq_sb[:], in_=qblob[:, 640:642]).then_inc(s_dve, 1)
            v.wait_ge(s_pe, 2)
            for h in range(2):
                v.tensor_scalar_add(out=hqs[:, h * 128:(h + 1) * 128],
                                    in0=qp[:, h * 128:(h + 1) * 128],
                                    scalar1=bq_sb[:, h:h + 1]).then_inc(s_dve, 1)
            v.wait_ge(s_pe, 4)
            v.tensor_copy(out=ms[:], in_=mp[:]).then_inc(s_dve, 1)
            v.wait_ge(s_pe, 22)  # all 4 z MMs done (counts 13,16,19,22)
            zv = zrow[:].rearrange("p (u h q) -> p h q u", u=2, h=2)
            for h in range(2):
                v.reduce_sum(out=zr_sb[:, h * 128:(h + 1) * 128], in_=zv[:, h],
                             axis=mybir.AxisListType.X).then_inc(s_dve, 1)
            v.wait_ge(s_pe, 25)
            v.reciprocal(out=ri0[:], in_=zc[:, 0:1]).then_inc(s_dve, 1)
            v.wait_ge(s_pe, 26)
            v.reciprocal(out=ri1[:], in_=zc[:, 1:2]).then_inc(s_dve, 1)
            v.wait_ge(s_pe, 27)
            v.tensor_scalar_mul(out=res0[:], in0=fin[:, 0:128],
                                scalar1=ri0[:, :1]).then_inc(s_dve, 1)
            v.wait_ge(s_act, 6)
            v.tensor_add(out=res0[:], in0=res0[:],
                         in1=res1[:]).then_inc(s_dve, 1)

        @block.tensor
        def _(t):
            t.wait_ge(dq, 16)
            for h in range(2):
                t.matmul(out=qp[:, h * 128:(h + 1) * 128],
                         lhsT=qblob[:, h * 128:(h + 1) * 128],
                         rhs=qblob[:, 256:384], start=True,
                         stop=True).then_inc(s_pe, 1)
            t.wait_ge(s_dve, 3)
            for h in range(2):
                t.matmul(out=mp[:, h * 128:(h + 1) * 128],
                         lhsT=qblob[:, 384 + h * 128:384 + (h + 1) * 128],
                         rhs=hqs[:, h * 128:(h + 1) * 128], start=True,
                         stop=True).then_inc(s_pe, 1)
            t.wait_ge(s_dve, 4)
            t.wait_ge(dk, 16)
            for c2 in range(4):
                if c2 >= 2:
                    t.wait_ge(s_act, c2 - 1)
                for j in range(2):
                    c = c2 * 2 + j
                    t.matmul(out=scs[c2 % 2][:, j * 256:(j + 1) * 256],
                             lhsT=kT[:, c * 128:(c + 1) * 128],
                             rhs=ms[:], start=True,
                             stop=True).then_inc(s_pe, 1)
            t.wait_ge(dx, 16)
            for c2 in range(4):
                t.wait_ge(s_act, c2 + 1)
                t.matmul(out=zrow[:], lhsT=ones_col[:],
                         rhs=pT[:, c2 * 512:(c2 + 1) * 512],
                         start=(c2 == 0), stop=(c2 == 3),
                         skip_group_check=True).then_inc(s_pe, 1)
                for j in range(2):
                    c = c2 * 2 + j
                    t.matmul(out=vo[:],
                             lhsT=xblob[:, c * 128:(c + 1) * 128],
                             rhs=pT[:, c * 256:(c + 1) * 256],
                             start=(c == 0), stop=(c == 7),
                             skip_group_check=True).then_inc(s_pe, 1)
            t.wait_ge(s_dve, 5)
            t.matmul(out=zc[:, 0:1], lhsT=zr_sb[:, 0:128], rhs=one11[:],
                     start=True, stop=True).then_inc(s_pe, 1)
            t.wait_ge(s_dve, 6)
            t.matmul(out=zc[:, 1:2], lhsT=zr_sb[:, 128:256], rhs=one11[:],
                     start=True, stop=True).then_inc(s_pe, 1)
            t.wait_ge(s_act, 5)
            for h in range(2):
                t.matmul(out=fin[:, h * 128:(h + 1) * 128],
                         lhsT=ot[:, h * 128:(h + 1) * 128],
                         rhs=xblob[:, T + h * LD:T + (h + 1) * LD],
                         start=True, stop=True).then_inc(s_pe, 1)

    st.close()
    nc.compile()
    return nc


USE_RAW = False


def _get_program(vd_name=None):
    vd_name = vd_name or VALUE_DTYPE
    key = ("raw" if USE_RAW else "tile") + vd_name
    if key not in _CACHE:
        builder = _build_program_raw if USE_RAW else _build_program
        _CACHE[key] = builder(vd_name)
    return _CACHE[key]


def _host_prep(ts, ys0, ys1, emb0, emb1):
    """Full k_in^T (permuted) per batch and q_in^T."""
    div = np.exp(np.arange(0, DT, 2, dtype=np.float32)
                 * (-np.log(10.0) / DT)).astype(np.float32)  # (32,)
    ang = 48.0 * ts[:, :, None].astype(np.float32) * div[None, None, :]
    kT = np.empty((N, KQ, T), np.float32)
    kT[:, 0:32] = np.sin(ang).transpose(0, 2, 1)
    kT[:, 32:64] = np.cos(ang).transpose(0, 2, 1)
    kT[:, 64:96] = emb0[ys0].transpose(0, 2, 1)
    kT[:, 96:128] = emb1[ys1].transpose(0, 2, 1)

    ref = np.linspace(0.0, 1.0, NREF, dtype=np.float32)
    ang_r = 48.0 * ref[:, None] * div[None, :]  # (NREF, 32)
    qT = np.empty((KQ, NREF), np.float32)
    qT[0:32] = np.sin(ang_r).T
    qT[32:64] = np.cos(ang_r).T
    qT[64:96] = emb0[100][:, None]
    qT[96:128] = emb1[50][:, None]
    return kT, qT


def _make_in_maps(ts, ys0, ys1, x, emb0, emb1, Wq, bq, Wk, bk, Wo, vd_name):
    if vd_name == "f16":
        sd = vd = np.float16
    elif vd_name == "bf16":
        sd = vd = ml_dtypes.bfloat16
    else:  # hybrid
        sd, vd = ml_dtypes.bfloat16, np.float32
    bf = sd
    ts = np.asarray(ts, np.float32)
    x = np.asarray(x, np.float32)
    emb0 = np.asarray(emb0, np.float32)
    emb1 = np.asarray(emb1, np.float32)
    ys0 = np.asarray(ys0).astype(np.int64)
    ys1 = np.asarray(ys1).astype(np.int64)

    kT, qT = _host_prep(ts, ys0, ys1, emb0, emb1)
    # KQ permutation: (sin block | cos block | emb0 | emb1) -> reference order
    perm = np.concatenate([2 * np.arange(32), 2 * np.arange(32) + 1,
                           64 + np.arange(32), 96 + np.arange(32)])
    Wq_p = np.asarray(Wq, np.float32)[perm]
    Wk_p = np.asarray(Wk, np.float32)[perm]
    bq2 = np.asarray(bq, np.float32).reshape(H, KQ)
    bk2 = np.asarray(bk, np.float32).reshape(H, KQ)
    Wo = np.asarray(Wo, np.float32)
    # x rearranged: chunk c on cols [c*128,(c+1)*128), key t=c*128+p on part p
    xr = np.ascontiguousarray(
        x.reshape(N, TCH, 128, LD).transpose(0, 2, 1, 3).reshape(N, 128, T))

    in_maps = []
    for c in range(NCORES):
        b, hg = c // 2, c % 2
        # wo laid out (LD, 2*LD): local head h rows at cols [h*LD,(h+1)*LD)
        wo2 = np.ascontiguousarray(
            Wo[hg * 256:(hg + 1) * 256, :].reshape(2, LD, LD)
            .transpose(1, 0, 2).reshape(LD, 2 * LD))
        ww = np.concatenate(
            [Wq_p[:, (2 * hg + h) * 128:(2 * hg + h + 1) * 128]
             @ Wk_p[:, (2 * hg + h) * 128:(2 * hg + h + 1) * 128].T
             for h in range(2)], axis=1)  # (KQ, 2*KQ): WW_h[e, c]
        wkbq = np.stack(
            [Wk_p[:, (2 * hg + h) * 128:(2 * hg + h + 1) * 128]
             @ bq2[2 * hg + h] for h in range(2)], axis=1)  # (KQ, 2)
        qblob = np.concatenate([ww, qT, wkbq], axis=1)
        xblob = np.concatenate([xr[b], wo2], axis=1)
        in_maps.append(dict(
            kT=kT[b].astype(bf),
            qblob=np.ascontiguousarray(qblob).astype(bf),
            xblob=np.ascontiguousarray(xblob).astype(vd),
        ))
    return in_maps


def kernel(ts, ys0, ys1, x, emb0, emb1, Wq, bq, Wk, bk, Wo, bo):
    in_maps = _make_in_maps(ts, ys0, ys1, x, emb0, emb1, Wq, bq, Wk, bk, Wo,
                            VALUE_DTYPE)
    nc = _get_program()
    res = run_bass_kernel_spmd(nc, in_maps, list(range(NCORES)))
    bo = np.asarray(bo, np.float32)
    out = np.empty((N, NREF, LD), np.float32)
    for b in range(N):
        out[b] = (res.results[2 * b]["out"] + res.results[2 * b + 1]["out"]
                  + bo[None, :])
    return out
